# revision 3
# baseline (speedup 1.0000x reference)
"""ALoraLinear on 8 TRN2 NeuronCores.

y = x @ W^T + b + mask ⊙ ((x @ A^T) @ B_w^T) * 2.0
  B=4, S=4096, D_IN=D_OUT=4096, R=32; mask = per-sample tail of the sequence.

Strategy: pure data-parallel over the 16384 flattened tokens (2048/core), no
collectives. Host-side prep (free w.r.t. HW time): transpose x and W into
partition-tiled [128, K/128, free] bf16 layouts, pad LoRA rank 32->128, fold
the bias into the LoRA matmul (constant-1 row 127 of the masked LoRA
activations x bias row 127 of padded B_w^T), fold mask*2.0 into a per-token
vector applied to the tiny LoRA activation u = A_pad @ x^T.

Per core: for each of 8x16 output tiles [128 tok, 512 dout], accumulate 32
K-tile matmuls of x^T·W^T plus one LoRA matmul into the same PSUM bank,
copy to SBUF on the vector engine, DMA out. Compute-bound at bf16
(~68.7 GFLOP/core vs 78.6 TFLOP/s peak).
"""

import numpy as np
import ml_dtypes

N_CORES = 8
B, S, D_IN, D_OUT, R = 4, 4096, 4096, 4096, 32
SCALING = 2.0
P = 128
TOKC = (B * S) // N_CORES  # 2048 tokens per core
KT = D_IN // P  # 32 k-tiles
NB = D_OUT // 512  # 8 n-blocks of 512
MT = TOKC // P  # 16 m-tiles of 128 tokens
NCHUNK = TOKC // 512  # 4 chunks for the LoRA activation

_COMPILED = None


def _build():
    import concourse.bacc as bacc
    import concourse.mybir as mybir
    import concourse.tile as tile

    bf16 = mybir.dt.bfloat16
    f32 = mybir.dt.float32

    nc = bacc.Bacc("TRN2", target_bir_lowering=False, debug=False)

    xt_d = nc.dram_tensor("xt", [P, KT, TOKC], bf16, kind="ExternalInput")
    wt_d = nc.dram_tensor("wt", [P, KT, D_OUT], bf16, kind="ExternalInput")
    at_d = nc.dram_tensor("at", [P, KT, P], bf16, kind="ExternalInput")
    bwt_d = nc.dram_tensor("bwt", [P, D_OUT], bf16, kind="ExternalInput")
    mask_d = nc.dram_tensor("mask", [P, TOKC], bf16, kind="ExternalInput")
    out_d = nc.dram_tensor("out", [TOKC, D_OUT], f32, kind="ExternalOutput")

    with tile.TileContext(nc) as tc:
        with (
            tc.tile_pool(name="const", bufs=1) as const,
            tc.tile_pool(name="xtp", bufs=1) as xtp,
            tc.tile_pool(name="utp", bufs=1) as utp,
            tc.tile_pool(name="wtp", bufs=KT) as wtp,
            tc.tile_pool(name="outp", bufs=3) as outp,
            tc.tile_pool(name="upsum", bufs=NCHUNK, space="PSUM") as upsum,
            tc.tile_pool(name="mpsum", bufs=4, space="PSUM") as mpsum,
        ):
            at_sb = const.tile([P, KT, P], bf16, name="at_sb")
            nc.sync.dma_start(at_sb[:], at_d.ap()[:])
            bwt_sb = const.tile([P, D_OUT], bf16, name="bwt_sb")
            nc.sync.dma_start(bwt_sb[:], bwt_d.ap()[:])
            mask_sb = const.tile([P, TOKC], bf16, name="mask_sb")
            nc.sync.dma_start(mask_sb[:], mask_d.ap()[:])

            xt_sb = xtp.tile([P, KT, TOKC], bf16, name="xt_sb")
            for k in range(KT):
                nc.sync.dma_start(xt_sb[:, k : k + 1, :], xt_d.ap()[:, k : k + 1, :])

            # LoRA activation u^T = A_pad @ x^T, masked & scaled, bf16.
            # Row 127 is forced to 1.0 so the bias row of bwt adds b.
            ut_sb = utp.tile([P, TOKC], bf16, name="ut_sb")
            ups = [upsum.tile([P, 512], f32, name="ups") for _ in range(NCHUNK)]
            for k in range(KT):
                for c in range(NCHUNK):
                    nc.tensor.matmul(
                        ups[c][:],
                        at_sb[:, k, :],
                        xt_sb[:, k, c * 512 : (c + 1) * 512],
                        start=(k == 0),
                        stop=(k == KT - 1),
                    )
            for c in range(NCHUNK):
                sl = slice(c * 512, (c + 1) * 512)
                nc.vector.tensor_mul(ut_sb[:, sl], ups[c][:], mask_sb[:, sl])
            # row 127 := 1.0 so the bias row of bwt contributes b. A compute
            # engine can't write a partition range starting at 127, so DMA
            # the host-provided ones row (mask row 127) over it.
            nc.sync.dma_start(ut_sb[127:128, :], mask_d.ap()[127:128, :])

            # Main matmul: W^T streamed by 512-wide n-blocks, cached across m.
            for n in range(NB):
                nsl = slice(n * 512, (n + 1) * 512)
                wt_tiles = []
                for k in range(KT):
                    wt = wtp.tile([P, 512], bf16, name="wt_sb")
                    nc.sync.dma_start(wt[:], wt_d.ap()[:, k, nsl])
                    wt_tiles.append(wt)
                for m in range(MT):
                    msl = slice(m * P, (m + 1) * P)
                    ps = mpsum.tile([P, 512], f32, name="ps")
                    for k in range(KT):
                        nc.tensor.matmul(
                            ps[:],
                            xt_sb[:, k, msl],
                            wt_tiles[k][:],
                            start=(k == 0),
                            stop=False,
                        )
                    nc.tensor.matmul(
                        ps[:], ut_sb[:, msl], bwt_sb[:, nsl], start=False, stop=True
                    )
                    ot = outp.tile([P, 512], f32, name="ot")
                    nc.vector.tensor_copy(ot[:], ps[:])
                    nc.sync.dma_start(out_d.ap()[msl, nsl], ot[:])

    nc.compile()
    return nc


def _get_compiled():
    global _COMPILED
    if _COMPILED is None:
        _COMPILED = _build()
    return _COMPILED


def _tile_kx(a_t: np.ndarray) -> np.ndarray:
    """[K, F] -> partition-tiled [128, K/128, F] bf16, C-contiguous."""
    k, f = a_t.shape
    return np.ascontiguousarray(
        a_t.reshape(k // P, P, f).transpose(1, 0, 2)
    ).astype(ml_dtypes.bfloat16)


def _prepare_in_maps(x, alora_offsets, W, b, A, B_w):
    bf = ml_dtypes.bfloat16
    xf = np.asarray(x, dtype=np.float32).reshape(B * S, D_IN)

    wt_np = _tile_kx(np.asarray(W, dtype=np.float32).T)  # [128, 32, 4096]

    A_pad = np.zeros((P, D_IN), dtype=np.float32)
    A_pad[:R] = np.asarray(A, dtype=np.float32)
    at_np = _tile_kx(A_pad.T)  # [128, 32, 128]

    bwt_np = np.zeros((P, D_OUT), dtype=np.float32)
    bwt_np[:R] = np.asarray(B_w, dtype=np.float32).T
    bwt_np[P - 1] = np.asarray(b, dtype=np.float32)  # bias row
    bwt_np = bwt_np.astype(bf)

    # per-token mask * SCALING over the flattened (b, s) axis
    offs = np.asarray(alora_offsets, dtype=np.int64)
    kk = np.minimum(offs, S)
    pos = np.arange(S, dtype=np.int64)
    mask_full = (pos[None, :] >= (S - kk)[:, None]).astype(np.float32) * SCALING
    mask_full = mask_full.reshape(B * S)

    in_maps = []
    for c in range(N_CORES):
        tok = slice(c * TOKC, (c + 1) * TOKC)
        xt_np = _tile_kx(xf[tok].T)  # [128, 32, 2048]
        mask_np = np.broadcast_to(mask_full[tok], (P, TOKC)).copy()
        mask_np[P - 1] = 1.0  # ones row, DMA'd into ut row 127 (bias path)
        mask_np = np.ascontiguousarray(mask_np).astype(bf)
        in_maps.append(
            {"xt": xt_np, "wt": wt_np, "at": at_np, "bwt": bwt_np, "mask": mask_np}
        )
    return in_maps


def _run(inputs: dict, trace: bool = False):
    from concourse.bass_utils import run_bass_kernel_spmd

    nc = _get_compiled()
    in_maps = _prepare_in_maps(**inputs)
    res = run_bass_kernel_spmd(
        nc, in_maps, core_ids=list(range(N_CORES)), trace=trace
    )
    out = np.concatenate(
        [res.results[c]["out"] for c in range(N_CORES)], axis=0
    ).reshape(B, S, D_OUT)
    return out, res


def kernel(x, alora_offsets, W, b, A, B_w) -> np.ndarray:
    out, _ = _run(
        {"x": x, "alora_offsets": alora_offsets, "W": W, "b": b, "A": A, "B_w": B_w}
    )
    return out
